# revision 18
# baseline (speedup 1.0000x reference)
"""Trainium2 Bass kernel for nn_ContrastLoss (LayerNorm + label segment-sum +
EMA codebook contrast loss), data-parallel over 8 NeuronCores.

Contract: kernel(**inputs) takes the FULL unsharded inputs
  input_f [128,1024,768] f32, char_dic [96,768] f32, ln_w [768] f32,
  ln_b [768] f32, target [128,1024] int64
and returns the full output (f32 scalar), matching reference.reference.

Strategy (hardcoded for the shapes above):
 - shard the batch dim over 8 cores: 16 batches = 16384 tokens per core
 - per core, stream 16 tiles of [128 partitions x 8 tokens x 768].
   The HBM->SBUF DMA casts f32->bf16 in flight (SWDGE), so no on-chip
   cast pass is needed.  Per half-tile (4 token groups):
     * ACT: one Square op [128,4,768] -> sq (bf16)
     * DVE: one multi-group reduce of xb -> sums[128,4], one of sq ->
       sumsq[128,4]
     * rstd = 1/sqrt((sumsq - sums^2/D)/D + eps) per token (small ops)
     * one-hot: (iota==label) as one bf16 tensor_tensor op, then
       scaled by rstd broadcast in a second op
     * TensorE: per token group 2 matmuls accumulate PSUM [96,384]x2;
       per half-tile one extra matmul ones^T @ oh_eq accumulates exact
       label counts into PSUM partition 96
   (ln_w/ln_b are folded out of the streaming loop entirely:
    tok_sums = (S - beta)*w + counts*b applied once at the end, where
    beta = row-mean of S recovers the -mu*rstd LayerNorm shift)
 - bf16 AllReduce of the [97,768] partial (S rows 0..95 | counts row 96)
   across cores; a tiny dummy AllReduce at kernel start absorbs the
   cross-core launch-skew barrier while the stream runs
 - tail math (group sums, positive term, EMA update, LayerNorm, negative
   term) computed replicated on every core; host reads core 0's scalar
"""

import os
import sys

for _p in ("/opt/trn_rl_repo",):
    if _p not in sys.path:
        sys.path.insert(0, _p)

import numpy as np
import ml_dtypes

import concourse.bass as bass
import concourse.bacc as bacc
import concourse.tile as tile
from concourse import mybir
from concourse.bass_utils import run_bass_kernel_spmd

F32 = mybir.dt.float32
BF16 = mybir.dt.bfloat16
AF = mybir.ActivationFunctionType
OP = mybir.AluOpType
AX = mybir.AxisListType

N_CORES = 8
B, S, D = 128, 1024, 768
NCHAR = 96
EPS = 1e-5
EMA = 0.1

TOK_PER_CORE = (B // N_CORES) * S          # 16384
T = 8                                      # tokens per partition per tile
H = T // 2                                 # tokens per half-tile
TILE_TOK = 128 * T                         # 1024 tokens per tile
N_TILES = TOK_PER_CORE // TILE_TOK         # 16


def build_kernel(n_tiles=N_TILES, trivial_wb=False):
    tok_per_core = n_tiles * TILE_TOK
    nc = bacc.Bacc("TRN2", target_bir_lowering=False, debug=False,
                   num_devices=N_CORES)

    x_d = nc.dram_tensor("x", [tok_per_core, D], F32, kind="ExternalInput")
    lab_d = nc.dram_tensor("lab", [128, n_tiles * T], BF16,
                           kind="ExternalInput")
    char_d = nc.dram_tensor("char", [NCHAR, D], F32, kind="ExternalInput")
    wbc_d = nc.dram_tensor("wbc", [NCHAR, D], F32, kind="ExternalInput")
    bbc_d = nc.dram_tensor("bbc", [NCHAR, D], F32, kind="ExternalInput")
    out_d = nc.dram_tensor("out", [1, 1], F32, kind="ExternalOutput")

    # constants embedded in the NEFF
    iota_np = np.tile(np.arange(NCHAR, dtype=np.float32), (128, T, 1))
    iota_d = nc.inline_tensor(iota_np.astype(ml_dtypes.bfloat16),
                              name="iotaH96")
    mask_np = np.ones((NCHAR, 1), dtype=np.float32)
    mask_np[0, 0] = 0.0
    mask_d = nc.inline_tensor(mask_np, name="maskrow")
    ones96_d = nc.inline_tensor(np.ones((NCHAR, 1), dtype=np.float32),
                                name="ones96")
    dmy_d = nc.inline_tensor(np.zeros((1, 128), dtype=np.float32),
                             name="dmyzero")

    # collective bounce buffers (bf16 payload: rows 0..95 = scaled segment
    # sums, row 96 cols 0..95 = exact label counts)
    cc_in = nc.dram_tensor("cc_in", [NCHAR + 1, D], BF16)
    cc_out = nc.dram_tensor("cc_out", [NCHAR + 1, D], BF16,
                            addr_space="Shared")
    dmy_out = nc.dram_tensor("dmy_out", [1, 128], F32, addr_space="Shared")

    x_r = x_d.ap().rearrange("(t p f) d -> t p f d",
                             t=n_tiles, p=128, f=T)

    with tile.TileContext(nc) as tc:
        with (
            nc.allow_low_precision(
                "bf16 row-sum outputs: DVE accumulates fp32 internally, "
                "only the final per-token value is downcast"),
            tc.tile_pool(name="consts", bufs=1) as consts,
            tc.tile_pool(name="xp", bufs=5) as xp,
            tc.tile_pool(name="sqp", bufs=3) as sqp,
            tc.tile_pool(name="trp", bufs=3) as trp,
            tc.tile_pool(name="stp", bufs=4) as stp,
            tc.tile_pool(name="ohp", bufs=3) as ohp,
            tc.tile_pool(name="tailp", bufs=1) as tailp,
            tc.tile_pool(name="psum", bufs=1, space="PSUM") as psp,
        ):
            # --- early dummy collective: pays the cross-core barrier /
            # bootstrap cost while the stream below runs ---
            nc.gpsimd.collective_compute(
                "AllReduce", OP.add,
                replica_groups=[list(range(N_CORES))],
                ins=[dmy_d.ap()], outs=[dmy_out.ap()],
            )

            # --- loop constants into SBUF ---
            iota_sb = consts.tile([128, T, NCHAR], BF16)
            nc.sync.dma_start(out=iota_sb[:], in_=iota_d.ap())
            ones128 = consts.tile([128, 1], BF16)
            nc.vector.memset(ones128[:], 1.0)
            eps128 = consts.tile([128, 1], F32)
            nc.vector.memset(eps128[:], EPS)
            eps96 = consts.tile([NCHAR, 1], F32)
            nc.vector.memset(eps96[:], EPS)
            # all labels for this core in one DMA: [p, tile*T]
            l_all = consts.tile([128, n_tiles * T], BF16)
            nc.sync.dma_start(out=l_all[:], in_=lab_d.ap())
            # tail constants (tiny, loaded while streaming)
            mask_sb = consts.tile([NCHAR, 1], F32)
            nc.sync.dma_start(out=mask_sb[:], in_=mask_d.ap())
            ones96_sb = consts.tile([NCHAR, 1], F32)
            nc.sync.dma_start(out=ones96_sb[:], in_=ones96_d.ap())
            char_sb = consts.tile([NCHAR, D], F32)
            nc.sync.dma_start(out=char_sb[:], in_=char_d.ap())
            if not trivial_wb:
                wbc_sb = consts.tile([NCHAR, D], F32)
                nc.sync.dma_start(out=wbc_sb[:], in_=wbc_d.ap())
                bbc_sb = consts.tile([NCHAR, D], F32)
                nc.sync.dma_start(out=bbc_sb[:], in_=bbc_d.ap())

            # --- PSUM accumulators for the streaming segment-sum ---
            psA = psp.tile([NCHAR, 384], F32)
            psB = psp.tile([NCHAR, 384], F32)
            cnt_ps = psp.tile([1, 384], F32)

            # --- streaming loop ---
            def add_tree(src, sums_out):
                """Row sums of src [128,T,768] via 2x-mode TT pairwise adds
                (DVE has no fast reduce: TENSOR_REDUCE is stuck at 1x)."""
                a1 = trp.tile([128, T, 384], BF16)
                nc.vector.tensor_tensor(a1[:], src[:, :, 0:384],
                                        src[:, :, 384:768], op=OP.add)
                a2 = trp.tile([128, T, 192], BF16)
                nc.vector.tensor_tensor(a2[:], a1[:, :, 0:192],
                                        a1[:, :, 192:384], op=OP.add)
                a3 = trp.tile([128, T, 96], BF16)
                nc.vector.tensor_tensor(a3[:], a2[:, :, 0:96],
                                        a2[:, :, 96:192], op=OP.add)
                nc.vector.reduce_sum(sums_out, a3[:], axis=AX.X)

            # 3-stage software pipeline: stage A(i) loads + row-stats,
            # stage B(i-1) finishes rstd, stage C(i-2) builds one-hots
            # and issues the matmuls.  The skew keeps every engine's
            # in-order instruction stream free of waits on ops issued in
            # the same iteration (ACT's sqrt / DVE's one-hot would
            # otherwise bubble both engines every half-tile).
            state = {}

            def stage_a(i):
                xb = xp.tile([128, T, D], BF16)
                sums = stp.tile([128, T], F32)
                sumsq = stp.tile([128, T], F32)
                for h in range(2):
                    g = slice(h * H, (h + 1) * H)
                    # f32 -> bf16 cast in flight (SWDGE)
                    nc.gpsimd.dma_start(out=xb[:, g, :],
                                        in_=x_r[i, :, g, :])
                add_tree(xb[:], sums[:])
                # row sums of squares: ~7/16 of tiles via ACT
                # accumulators, the rest via batched ACT Square +
                # DVE add-tree (engine balance)
                if (i % 16) in (1, 3, 5, 7, 9, 11, 13):
                    trash = sqp.tile([128, T, D], BF16)
                    for t in range(T):
                        nc.scalar.activation(
                            trash[:, t, :], xb[:, t, :], AF.Square,
                            accum_out=sumsq[:, t:t + 1])
                else:
                    sq = sqp.tile([128, T, D], BF16)
                    nc.scalar.activation(sq[:], xb[:], AF.Square)
                    add_tree(sq[:], sumsq[:])
                state[i] = {"xb": xb, "sums": sums, "sumsq": sumsq}

            def stage_b(i):
                st = state[i]
                # rstd: 1/sqrt((sumsq - sums^2/D)/D + eps)
                s2 = stp.tile([128, T], F32)
                nc.vector.scalar_tensor_tensor(
                    s2[:], st["sums"][:], 1.0 / D, st["sums"][:],
                    OP.mult, OP.mult)
                xv = stp.tile([128, T], F32)
                nc.vector.tensor_sub(xv[:], st["sumsq"][:], s2[:])
                stdv = stp.tile([128, T], F32)
                nc.scalar.activation(stdv[:], xv[:], AF.Sqrt,
                                     bias=eps128[:], scale=1.0 / D)
                rstd = stp.tile([128, T], F32)
                nc.vector.reciprocal(rstd[:], stdv[:])
                st["rstd"] = rstd

            def stage_c(i):
                st = state.pop(i)
                xb, rstd = st["xb"], st["rstd"]
                oh_eq = ohp.tile([128, T, NCHAR], BF16)
                lab_b = l_all[:, i * T:(i + 1) * T]
                nc.vector.tensor_tensor(
                    oh_eq[:], iota_sb[:],
                    lab_b[:, :, None].broadcast_to((128, T, NCHAR)),
                    op=OP.is_equal)
                oh = ohp.tile([128, T, NCHAR], BF16)
                nc.vector.tensor_tensor(
                    oh[:], oh_eq[:],
                    rstd[:, :, None].broadcast_to((128, T, NCHAR)),
                    op=OP.mult)

                for t in range(T):
                    st0 = i == 0 and t == 0
                    sp0 = i == n_tiles - 1 and t == T - 1
                    nc.tensor.matmul(psA[:], oh[:, t, :],
                                     xb[:, t, 0:384],
                                     start=st0, stop=sp0)
                    nc.tensor.matmul(psB[:], oh[:, t, :],
                                     xb[:, t, 384:D],
                                     start=st0, stop=sp0)
                first, last = i == 0, i == n_tiles - 1
                nc.tensor.matmul(cnt_ps[:], ones128[:],
                                 oh_eq[:, 0:H, :],
                                 start=first, stop=last)
                nc.tensor.matmul(cnt_ps[:], ones128[:],
                                 oh_eq[:, H:T, :],
                                 start=False if not first else False,
                                 stop=last)

            # issue order C, B, A: the one-hot TTs lead each iteration's
            # DVE stream, so TensorE gets its matmul inputs at the start
            # of the period instead of after the next tile's add-trees
            for i in range(n_tiles + 2):
                if 0 <= i - 2 < n_tiles:
                    stage_c(i - 2)
                if 0 <= i - 1 < n_tiles:
                    stage_b(i - 1)
                if i < n_tiles:
                    stage_a(i)

            # --- local partials -> bf16 -> DRAM -> AllReduce ---
            acc = tailp.tile([NCHAR, D], BF16)
            nc.vector.tensor_copy(acc[:, 0:384], psA[:])
            nc.vector.tensor_copy(acc[:, 384:D], psB[:])
            # counts: cnt_ps holds [u*96+c] = counts of groups (u, 4+u);
            # fold the 4 u-slices, then ship as payload row 96
            cnt_pay = tailp.tile([1, D], BF16)
            nc.vector.memset(cnt_pay[:], 0.0)
            cnt_red = tailp.tile([1, NCHAR], F32)
            nc.vector.reduce_sum(
                cnt_red[:],
                cnt_ps[:].rearrange("p (t c) -> p c t", c=NCHAR),
                axis=AX.X)
            nc.vector.tensor_copy(cnt_pay[:, 0:NCHAR], cnt_red[:])
            nc.sync.dma_start(out=cc_in.ap()[0:NCHAR, :], in_=acc[:])
            nc.sync.dma_start(out=cc_in.ap()[NCHAR:NCHAR + 1, :],
                              in_=cnt_pay[:])
            nc.gpsimd.collective_compute(
                "AllReduce", OP.add,
                replica_groups=[list(range(N_CORES))],
                ins=[cc_in.ap()], outs=[cc_out.ap()],
            )
            redb = tailp.tile([NCHAR, D], BF16)
            nc.sync.dma_start(out=redb[:], in_=cc_out.ap()[0:NCHAR, :])
            cntb = tailp.tile([NCHAR, 1], BF16)
            nc.sync.dma_start(
                out=cntb[:],
                in_=cc_out.ap()[NCHAR:NCHAR + 1, 0:NCHAR].rearrange(
                    "p c -> c p"))
            cnt = tailp.tile([NCHAR, 1], F32)
            nc.vector.tensor_copy(cnt[:], cntb[:])

            # beta_i = mean_d S[i, d]  (the LayerNorm -mu*rstd correction
            # folds into a row-mean of the scaled segment sums)
            rs = tailp.tile([NCHAR, 1], F32)
            nc.vector.reduce_sum(rs[:], redb[:], axis=AX.X)
            nb = tailp.tile([NCHAR, 1], F32)
            nc.vector.tensor_scalar(nb[:], rs[:], -1.0 / D, None, OP.mult)
            # group_sum = char + (S - beta)*w + counts*b
            group = tailp.tile([NCHAR, D], F32)
            if trivial_wb:
                nc.vector.scalar_tensor_tensor(group[:], redb[:], nb[:],
                                               char_sb[:], OP.add, OP.add)
            else:
                tmp1 = tailp.tile([NCHAR, D], F32)
                nc.vector.scalar_tensor_tensor(tmp1[:], bbc_sb[:], cnt[:],
                                               char_sb[:], OP.mult, OP.add)
                nc.vector.scalar_tensor_tensor(group[:], redb[:], nb[:],
                                               wbc_sb[:], OP.add, OP.mult)
                nc.vector.tensor_add(group[:], group[:], tmp1[:])

            # positive = sum(group^2) (divide by D at the very end)
            sqg = tailp.tile([NCHAR, D], F32)
            pos_col = tailp.tile([NCHAR, 1], F32)
            nc.scalar.activation(sqg[:], group[:], AF.Square,
                                 accum_out=pos_col[:])
            pos_ps = psp.tile([1, 1], F32)
            nc.tensor.matmul(pos_ps[:], ones96_sb[:], pos_col[:],
                             start=True, stop=True)
            pos_sb = tailp.tile([1, 1], F32)
            nc.vector.tensor_copy(pos_sb[:], pos_ps[:])

            # EMA update: new_char = char + 0.1 * group/(counts+1); row 0 kept
            cnt1 = tailp.tile([NCHAR, 1], F32)
            nc.vector.tensor_scalar(cnt1[:], cnt[:], 1.0, None, OP.add)
            invc = tailp.tile([NCHAR, 1], F32)
            nc.vector.reciprocal(invc[:], cnt1[:])
            ema = tailp.tile([NCHAR, D], F32)
            nc.vector.tensor_scalar(ema[:], group[:], invc[:], EMA,
                                    OP.mult, OP.mult)
            newc = tailp.tile([NCHAR, D], F32)
            nc.vector.tensor_add(newc[:], char_sb[:], ema[:])
            nc.vector.tensor_copy(newc[0:1, :], char_sb[0:1, :])

            # LayerNorm(new_char) with w/b
            bn2 = tailp.tile([NCHAR, 2, 6], F32)
            for gidx in range(2):
                nc.vector.bn_stats(bn2[:, gidx, :],
                                   newc[:, gidx * 384:(gidx + 1) * 384])
            st2 = tailp.tile([NCHAR, 2], F32)
            nc.vector.bn_aggr(st2[:], bn2[:])
            std2 = tailp.tile([NCHAR, 1], F32)
            nc.scalar.activation(std2[:], st2[:, 1:2], AF.Sqrt,
                                 bias=eps96[:], scale=1.0)
            rstd2 = tailp.tile([NCHAR, 1], F32)
            nc.vector.reciprocal(rstd2[:], std2[:])
            nmr2 = tailp.tile([NCHAR, 1], F32)
            nc.vector.scalar_tensor_tensor(nmr2[:], st2[:, 0:1], -1.0,
                                           rstd2[:], OP.mult, OP.mult)
            nrm = tailp.tile([NCHAR, D], F32)
            nc.scalar.activation(nrm[:], newc[:], AF.Identity,
                                 bias=nmr2[:], scale=rstd2[:])
            if trivial_wb:
                fin = nrm
            else:
                fin = tailp.tile([NCHAR, D], F32)
                nc.vector.tensor_mul(fin[:], nrm[:], wbc_sb[:])
                nc.vector.tensor_add(fin[:], fin[:], bbc_sb[:])

            # s = sum over rows 1..95 -> [1,768]; negative = sum(s^2)
            sA = psp.tile([1, 384], F32)
            sB = psp.tile([1, 384], F32)
            nc.tensor.matmul(sA[:], mask_sb[:], fin[:, 0:384],
                             start=True, stop=True)
            nc.tensor.matmul(sB[:], mask_sb[:], fin[:, 384:D],
                             start=True, stop=True)
            sqA = tailp.tile([1, 384], F32)
            sqB = tailp.tile([1, 384], F32)
            negA = tailp.tile([1, 1], F32)
            negB = tailp.tile([1, 1], F32)
            nc.scalar.activation(sqA[:], sA[:], AF.Square, accum_out=negA[:])
            nc.scalar.activation(sqB[:], sB[:], AF.Square, accum_out=negB[:])

            res = tailp.tile([1, 1], F32)
            nc.vector.tensor_add(res[:], negA[:], negB[:])
            nc.vector.tensor_sub(res[:], res[:], pos_sb[:])
            nc.vector.tensor_scalar(res[:], res[:], 1.0 / D, None, OP.mult)
            nc.sync.dma_start(out=out_d.ap(), in_=res[:])

    nc.finalize()
    return nc


_NC_CACHE = {}


def _get_nc(trivial_wb):
    if trivial_wb not in _NC_CACHE:
        _NC_CACHE[trivial_wb] = build_kernel(trivial_wb=trivial_wb)
    return _NC_CACHE[trivial_wb]


def make_in_maps(input_f, char_dic, ln_w, ln_b, target):
    input_f = np.ascontiguousarray(np.asarray(input_f, dtype=np.float32))
    char_dic = np.ascontiguousarray(np.asarray(char_dic, dtype=np.float32))
    ln_w = np.asarray(ln_w, dtype=np.float32)
    ln_b = np.asarray(ln_b, dtype=np.float32)
    labels = np.asarray(target).reshape(B, S)

    wbc = np.ascontiguousarray(np.broadcast_to(ln_w[None, :], (NCHAR, D)))
    bbc = np.ascontiguousarray(np.broadcast_to(ln_b[None, :], (NCHAR, D)))

    bpc = B // N_CORES
    in_maps = []
    for c in range(N_CORES):
        x_c = input_f[c * bpc:(c + 1) * bpc].reshape(TOK_PER_CORE, D)
        l_c = labels[c * bpc:(c + 1) * bpc].reshape(TOK_PER_CORE)
        # [tok] -> [p, tile*T]: token (i, p, f) lives at labT[p, i*T+f]
        l_t = np.ascontiguousarray(
            l_c.reshape(N_TILES, 128, T).transpose(1, 0, 2)
            .reshape(128, N_TILES * T).astype(ml_dtypes.bfloat16))
        in_maps.append({
            "x": np.ascontiguousarray(x_c),
            "lab": l_t,
            "char": char_dic,
            "wbc": wbc,
            "bbc": bbc,
        })
    return in_maps


def run(trace=False, **inputs):
    trivial_wb = bool(
        np.all(np.asarray(inputs["ln_w"], dtype=np.float32) == 1.0)
        and np.all(np.asarray(inputs["ln_b"], dtype=np.float32) == 0.0))
    nc = _get_nc(trivial_wb)
    in_maps = make_in_maps(**inputs)
    res = run_bass_kernel_spmd(nc, in_maps, core_ids=list(range(N_CORES)),
                               trace=trace)
    out = np.float32(res.results[0]["out"][0, 0])
    return out, res


def kernel(**inputs):
    out, _ = run(trace=False, **inputs)
    return np.array(out, dtype=np.float32)


if __name__ == "__main__":
    np.random.seed(0)
    input_f = np.random.randn(B, S, D).astype(np.float32)
    char_dic = np.random.randn(NCHAR, D).astype(np.float32)
    ln_w = np.ones(D, np.float32)
    ln_b = np.zeros(D, np.float32)
    target = np.random.randint(0, NCHAR, (B, S)).astype(np.int64)
    out = kernel(input_f=input_f, char_dic=char_dic, ln_w=ln_w,
                 ln_b=ln_b, target=target)
    print("kernel out:", out)


# revision 19
# speedup vs baseline: 1.0478x; 1.0478x over previous
"""Trainium2 Bass kernel for nn_ContrastLoss (LayerNorm + label segment-sum +
EMA codebook contrast loss), data-parallel over 8 NeuronCores.

Contract: kernel(**inputs) takes the FULL unsharded inputs
  input_f [128,1024,768] f32, char_dic [96,768] f32, ln_w [768] f32,
  ln_b [768] f32, target [128,1024] int64
and returns the full output (f32 scalar), matching reference.reference.

Strategy (hardcoded for the shapes above):
 - shard the batch dim over 8 cores: 16 batches = 16384 tokens per core
 - per core, stream 16 tiles of [128 partitions x 8 tokens x 768].
   The HBM->SBUF DMA casts f32->bf16 in flight (SWDGE), so no on-chip
   cast pass is needed.  Per half-tile (4 token groups):
     * ACT: one Square op [128,4,768] -> sq (bf16)
     * DVE: one multi-group reduce of xb -> sums[128,4], one of sq ->
       sumsq[128,4]
     * rstd = 1/sqrt((sumsq - sums^2/D)/D + eps) per token (small ops)
     * one-hot: (iota==label) as one bf16 tensor_tensor op, then
       scaled by rstd broadcast in a second op
     * TensorE: per token group 2 matmuls accumulate PSUM [96,384]x2;
       per half-tile one extra matmul ones^T @ oh_eq accumulates exact
       label counts into PSUM partition 96
   (ln_w/ln_b are folded out of the streaming loop entirely:
    tok_sums = (S - beta)*w + counts*b applied once at the end, where
    beta = row-mean of S recovers the -mu*rstd LayerNorm shift)
 - bf16 AllReduce of the [97,768] partial (S rows 0..95 | counts row 96)
   across cores; a tiny dummy AllReduce at kernel start absorbs the
   cross-core launch-skew barrier while the stream runs
 - tail math (group sums, positive term, EMA update, LayerNorm, negative
   term) computed replicated on every core; host reads core 0's scalar
"""

import os
import sys

for _p in ("/opt/trn_rl_repo",):
    if _p not in sys.path:
        sys.path.insert(0, _p)

import numpy as np
import ml_dtypes

import concourse.bass as bass
import concourse.bacc as bacc
import concourse.tile as tile
from concourse import mybir
from concourse.bass_utils import run_bass_kernel_spmd

F32 = mybir.dt.float32
BF16 = mybir.dt.bfloat16
AF = mybir.ActivationFunctionType
OP = mybir.AluOpType
AX = mybir.AxisListType

N_CORES = 8
B, S, D = 128, 1024, 768
NCHAR = 96
EPS = 1e-5
EMA = 0.1

TOK_PER_CORE = (B // N_CORES) * S          # 16384
T = 8                                      # tokens per partition per tile
H = T // 2                                 # tokens per half-tile
TILE_TOK = 128 * T                         # 1024 tokens per tile
N_TILES = TOK_PER_CORE // TILE_TOK         # 16


def build_kernel(n_tiles=N_TILES, trivial_wb=False):
    tok_per_core = n_tiles * TILE_TOK
    nc = bacc.Bacc("TRN2", target_bir_lowering=False, debug=False,
                   num_devices=N_CORES)

    x_d = nc.dram_tensor("x", [tok_per_core, D], F32, kind="ExternalInput")
    lab_d = nc.dram_tensor("lab", [128, n_tiles * T], BF16,
                           kind="ExternalInput")
    char_d = nc.dram_tensor("char", [NCHAR, D], F32, kind="ExternalInput")
    wbc_d = nc.dram_tensor("wbc", [NCHAR, D], F32, kind="ExternalInput")
    bbc_d = nc.dram_tensor("bbc", [NCHAR, D], F32, kind="ExternalInput")
    out_d = nc.dram_tensor("out", [1, 1], F32, kind="ExternalOutput")

    # constants embedded in the NEFF
    iota_np = np.tile(np.arange(NCHAR, dtype=np.float32), (128, T, 1))
    iota_d = nc.inline_tensor(iota_np.astype(ml_dtypes.bfloat16),
                              name="iotaH96")
    mask_np = np.ones((NCHAR, 1), dtype=np.float32)
    mask_np[0, 0] = 0.0
    mask_d = nc.inline_tensor(mask_np, name="maskrow")
    ones96_d = nc.inline_tensor(np.ones((NCHAR, 1), dtype=np.float32),
                                name="ones96")
    dmy_d = nc.inline_tensor(np.zeros((1, 128), dtype=np.float32),
                             name="dmyzero")

    # collective bounce buffers (bf16 payload: rows 0..95 = scaled segment
    # sums, row 96 cols 0..95 = exact label counts)
    cc_in = nc.dram_tensor("cc_in", [NCHAR + 1, D], BF16)
    cc_out = nc.dram_tensor("cc_out", [NCHAR + 1, D], BF16,
                            addr_space="Shared")
    dmy_out = nc.dram_tensor("dmy_out", [1, 128], F32, addr_space="Shared")

    x_r = x_d.ap().rearrange("(t p f) d -> t p f d",
                             t=n_tiles, p=128, f=T)

    with tile.TileContext(nc) as tc:
        with (
            nc.allow_low_precision(
                "bf16 row-sum outputs: DVE accumulates fp32 internally, "
                "only the final per-token value is downcast"),
            tc.tile_pool(name="consts", bufs=1) as consts,
            tc.tile_pool(name="xp", bufs=5) as xp,
            tc.tile_pool(name="sqp", bufs=3) as sqp,
            tc.tile_pool(name="trp", bufs=3) as trp,
            tc.tile_pool(name="stp", bufs=4) as stp,
            tc.tile_pool(name="ohp", bufs=3) as ohp,
            tc.tile_pool(name="tailp", bufs=1) as tailp,
            tc.tile_pool(name="psum", bufs=1, space="PSUM") as psp,
        ):
            # --- early dummy collective: pays the cross-core barrier /
            # bootstrap cost while the stream below runs ---
            nc.gpsimd.collective_compute(
                "AllReduce", OP.add,
                replica_groups=[list(range(N_CORES))],
                ins=[dmy_d.ap()], outs=[dmy_out.ap()],
            )

            # --- loop constants into SBUF ---
            iota_sb = consts.tile([128, T, NCHAR], BF16)
            nc.sync.dma_start(out=iota_sb[:], in_=iota_d.ap())
            ones128 = consts.tile([128, 1], BF16)
            nc.vector.memset(ones128[:], 1.0)
            eps128 = consts.tile([128, 1], F32)
            nc.vector.memset(eps128[:], EPS)
            eps96 = consts.tile([NCHAR, 1], F32)
            nc.vector.memset(eps96[:], EPS)
            # all labels for this core in one DMA: [p, tile*T]
            l_all = consts.tile([128, n_tiles * T], BF16)
            nc.sync.dma_start(out=l_all[:], in_=lab_d.ap())
            # tail constants (tiny, loaded while streaming)
            mask_sb = consts.tile([NCHAR, 1], F32)
            nc.sync.dma_start(out=mask_sb[:], in_=mask_d.ap())
            ones96_sb = consts.tile([NCHAR, 1], F32)
            nc.sync.dma_start(out=ones96_sb[:], in_=ones96_d.ap())
            char_sb = consts.tile([NCHAR, D], F32)
            nc.sync.dma_start(out=char_sb[:], in_=char_d.ap())
            if not trivial_wb:
                wbc_sb = consts.tile([NCHAR, D], F32)
                nc.sync.dma_start(out=wbc_sb[:], in_=wbc_d.ap())
                bbc_sb = consts.tile([NCHAR, D], F32)
                nc.sync.dma_start(out=bbc_sb[:], in_=bbc_d.ap())

            # --- PSUM accumulators for the streaming segment-sum ---
            psA = psp.tile([NCHAR, 384], F32)
            psB = psp.tile([NCHAR, 384], F32)
            cnt_ps = psp.tile([1, 384], F32)

            # --- streaming loop ---
            def add_tree(src, sums_out, n):
                """Row sums of src [128,n,768] via 2x-mode TT pairwise adds
                (DVE has no fast reduce: TENSOR_REDUCE is stuck at 1x)."""
                a1 = trp.tile([128, n, 384], BF16)
                nc.vector.tensor_tensor(a1[:], src[:, :, 0:384],
                                        src[:, :, 384:768], op=OP.add)
                a2 = trp.tile([128, n, 192], BF16)
                nc.vector.tensor_tensor(a2[:], a1[:, :, 0:192],
                                        a1[:, :, 192:384], op=OP.add)
                a3 = trp.tile([128, n, 96], BF16)
                nc.vector.tensor_tensor(a3[:], a2[:, :, 0:96],
                                        a2[:, :, 96:192], op=OP.add)
                nc.vector.reduce_sum(sums_out, a3[:], axis=AX.X)

            # 3-stage software pipeline: stage A(i) loads + row-stats,
            # stage B(i-1) finishes rstd, stage C(i-2) builds one-hots
            # and issues the matmuls.  The skew keeps every engine's
            # in-order instruction stream free of waits on ops issued in
            # the same iteration (ACT's sqrt / DVE's one-hot would
            # otherwise bubble both engines every half-tile).
            state = {}

            def stage_a(i):
                xb = xp.tile([128, T, D], BF16)
                sums = stp.tile([128, T], F32)
                sumsq = stp.tile([128, T], F32)
                for h in range(2):
                    g = slice(h * H, (h + 1) * H)
                    # f32 -> bf16 cast in flight (SWDGE)
                    nc.gpsimd.dma_start(out=xb[:, g, :],
                                        in_=x_r[i, :, g, :])
                add_tree(xb[:], sums[:], T)
                # row sums of squares, split within every tile so each
                # iteration loads ACT and DVE identically: the first 4
                # groups use ACT Square+accumulator, the last 4 use one
                # batched ACT Square + a DVE add-tree
                K = 4
                trash = sqp.tile([128, K, D], BF16)
                for t in range(K):
                    nc.scalar.activation(
                        trash[:, t, :], xb[:, t, :], AF.Square,
                        accum_out=sumsq[:, t:t + 1])
                sq = sqp.tile([128, T - K, D], BF16)
                nc.scalar.activation(sq[:], xb[:, K:T, :], AF.Square)
                add_tree(sq[:], sumsq[:, K:T], T - K)
                state[i] = {"xb": xb, "sums": sums, "sumsq": sumsq}

            def stage_b(i):
                st = state[i]
                # rstd: 1/sqrt((sumsq - sums^2/D)/D + eps)
                s2 = stp.tile([128, T], F32)
                nc.vector.scalar_tensor_tensor(
                    s2[:], st["sums"][:], 1.0 / D, st["sums"][:],
                    OP.mult, OP.mult)
                xv = stp.tile([128, T], F32)
                nc.vector.tensor_sub(xv[:], st["sumsq"][:], s2[:])
                stdv = stp.tile([128, T], F32)
                nc.scalar.activation(stdv[:], xv[:], AF.Sqrt,
                                     bias=eps128[:], scale=1.0 / D)
                rstd = stp.tile([128, T], F32)
                nc.vector.reciprocal(rstd[:], stdv[:])
                st["rstd"] = rstd

            def stage_c(i):
                st = state.pop(i)
                xb, rstd = st["xb"], st["rstd"]
                oh_eq = ohp.tile([128, T, NCHAR], BF16)
                lab_b = l_all[:, i * T:(i + 1) * T]
                nc.vector.tensor_tensor(
                    oh_eq[:], iota_sb[:],
                    lab_b[:, :, None].broadcast_to((128, T, NCHAR)),
                    op=OP.is_equal)
                oh = ohp.tile([128, T, NCHAR], BF16)
                nc.vector.tensor_tensor(
                    oh[:], oh_eq[:],
                    rstd[:, :, None].broadcast_to((128, T, NCHAR)),
                    op=OP.mult)

                for t in range(T):
                    st0 = i == 0 and t == 0
                    sp0 = i == n_tiles - 1 and t == T - 1
                    nc.tensor.matmul(psA[:], oh[:, t, :],
                                     xb[:, t, 0:384],
                                     start=st0, stop=sp0)
                    nc.tensor.matmul(psB[:], oh[:, t, :],
                                     xb[:, t, 384:D],
                                     start=st0, stop=sp0)
                first, last = i == 0, i == n_tiles - 1
                nc.tensor.matmul(cnt_ps[:], ones128[:],
                                 oh_eq[:, 0:H, :],
                                 start=first, stop=last)
                nc.tensor.matmul(cnt_ps[:], ones128[:],
                                 oh_eq[:, H:T, :],
                                 start=False if not first else False,
                                 stop=last)

            # issue order C, B, A: the one-hot TTs lead each iteration's
            # DVE stream, so TensorE gets its matmul inputs at the start
            # of the period instead of after the next tile's add-trees
            for i in range(n_tiles + 2):
                if 0 <= i - 2 < n_tiles:
                    stage_c(i - 2)
                if 0 <= i - 1 < n_tiles:
                    stage_b(i - 1)
                if i < n_tiles:
                    stage_a(i)

            # --- local partials -> bf16 -> DRAM -> AllReduce ---
            acc = tailp.tile([NCHAR, D], BF16)
            nc.vector.tensor_copy(acc[:, 0:384], psA[:])
            nc.vector.tensor_copy(acc[:, 384:D], psB[:])
            # counts: cnt_ps holds [u*96+c] = counts of groups (u, 4+u);
            # fold the 4 u-slices, then ship as payload row 96
            cnt_pay = tailp.tile([1, D], BF16)
            nc.vector.memset(cnt_pay[:], 0.0)
            cnt_red = tailp.tile([1, NCHAR], F32)
            nc.vector.reduce_sum(
                cnt_red[:],
                cnt_ps[:].rearrange("p (t c) -> p c t", c=NCHAR),
                axis=AX.X)
            nc.vector.tensor_copy(cnt_pay[:, 0:NCHAR], cnt_red[:])
            nc.sync.dma_start(out=cc_in.ap()[0:NCHAR, :], in_=acc[:])
            nc.sync.dma_start(out=cc_in.ap()[NCHAR:NCHAR + 1, :],
                              in_=cnt_pay[:])
            nc.gpsimd.collective_compute(
                "AllReduce", OP.add,
                replica_groups=[list(range(N_CORES))],
                ins=[cc_in.ap()], outs=[cc_out.ap()],
            )
            redb = tailp.tile([NCHAR, D], BF16)
            nc.sync.dma_start(out=redb[:], in_=cc_out.ap()[0:NCHAR, :])
            cntb = tailp.tile([NCHAR, 1], BF16)
            nc.sync.dma_start(
                out=cntb[:],
                in_=cc_out.ap()[NCHAR:NCHAR + 1, 0:NCHAR].rearrange(
                    "p c -> c p"))
            cnt = tailp.tile([NCHAR, 1], F32)
            nc.vector.tensor_copy(cnt[:], cntb[:])

            # beta_i = mean_d S[i, d]  (the LayerNorm -mu*rstd correction
            # folds into a row-mean of the scaled segment sums)
            rs = tailp.tile([NCHAR, 1], F32)
            nc.vector.reduce_sum(rs[:], redb[:], axis=AX.X)
            nb = tailp.tile([NCHAR, 1], F32)
            nc.vector.tensor_scalar(nb[:], rs[:], -1.0 / D, None, OP.mult)
            # group_sum = char + (S - beta)*w + counts*b
            group = tailp.tile([NCHAR, D], F32)
            if trivial_wb:
                nc.vector.scalar_tensor_tensor(group[:], redb[:], nb[:],
                                               char_sb[:], OP.add, OP.add)
            else:
                tmp1 = tailp.tile([NCHAR, D], F32)
                nc.vector.scalar_tensor_tensor(tmp1[:], bbc_sb[:], cnt[:],
                                               char_sb[:], OP.mult, OP.add)
                nc.vector.scalar_tensor_tensor(group[:], redb[:], nb[:],
                                               wbc_sb[:], OP.add, OP.mult)
                nc.vector.tensor_add(group[:], group[:], tmp1[:])

            # positive = sum(group^2) (divide by D at the very end)
            sqg = tailp.tile([NCHAR, D], F32)
            pos_col = tailp.tile([NCHAR, 1], F32)
            nc.scalar.activation(sqg[:], group[:], AF.Square,
                                 accum_out=pos_col[:])
            pos_ps = psp.tile([1, 1], F32)
            nc.tensor.matmul(pos_ps[:], ones96_sb[:], pos_col[:],
                             start=True, stop=True)
            pos_sb = tailp.tile([1, 1], F32)
            nc.vector.tensor_copy(pos_sb[:], pos_ps[:])

            # EMA update: new_char = char + 0.1 * group/(counts+1); row 0 kept
            cnt1 = tailp.tile([NCHAR, 1], F32)
            nc.vector.tensor_scalar(cnt1[:], cnt[:], 1.0, None, OP.add)
            invc = tailp.tile([NCHAR, 1], F32)
            nc.vector.reciprocal(invc[:], cnt1[:])
            ema = tailp.tile([NCHAR, D], F32)
            nc.vector.tensor_scalar(ema[:], group[:], invc[:], EMA,
                                    OP.mult, OP.mult)
            newc = tailp.tile([NCHAR, D], F32)
            nc.vector.tensor_add(newc[:], char_sb[:], ema[:])
            nc.vector.tensor_copy(newc[0:1, :], char_sb[0:1, :])

            # LayerNorm(new_char) with w/b
            bn2 = tailp.tile([NCHAR, 2, 6], F32)
            for gidx in range(2):
                nc.vector.bn_stats(bn2[:, gidx, :],
                                   newc[:, gidx * 384:(gidx + 1) * 384])
            st2 = tailp.tile([NCHAR, 2], F32)
            nc.vector.bn_aggr(st2[:], bn2[:])
            std2 = tailp.tile([NCHAR, 1], F32)
            nc.scalar.activation(std2[:], st2[:, 1:2], AF.Sqrt,
                                 bias=eps96[:], scale=1.0)
            rstd2 = tailp.tile([NCHAR, 1], F32)
            nc.vector.reciprocal(rstd2[:], std2[:])
            nmr2 = tailp.tile([NCHAR, 1], F32)
            nc.vector.scalar_tensor_tensor(nmr2[:], st2[:, 0:1], -1.0,
                                           rstd2[:], OP.mult, OP.mult)
            nrm = tailp.tile([NCHAR, D], F32)
            nc.scalar.activation(nrm[:], newc[:], AF.Identity,
                                 bias=nmr2[:], scale=rstd2[:])
            if trivial_wb:
                fin = nrm
            else:
                fin = tailp.tile([NCHAR, D], F32)
                nc.vector.tensor_mul(fin[:], nrm[:], wbc_sb[:])
                nc.vector.tensor_add(fin[:], fin[:], bbc_sb[:])

            # s = sum over rows 1..95 -> [1,768]; negative = sum(s^2)
            sA = psp.tile([1, 384], F32)
            sB = psp.tile([1, 384], F32)
            nc.tensor.matmul(sA[:], mask_sb[:], fin[:, 0:384],
                             start=True, stop=True)
            nc.tensor.matmul(sB[:], mask_sb[:], fin[:, 384:D],
                             start=True, stop=True)
            sqA = tailp.tile([1, 384], F32)
            sqB = tailp.tile([1, 384], F32)
            negA = tailp.tile([1, 1], F32)
            negB = tailp.tile([1, 1], F32)
            nc.scalar.activation(sqA[:], sA[:], AF.Square, accum_out=negA[:])
            nc.scalar.activation(sqB[:], sB[:], AF.Square, accum_out=negB[:])

            res = tailp.tile([1, 1], F32)
            nc.vector.tensor_add(res[:], negA[:], negB[:])
            nc.vector.tensor_sub(res[:], res[:], pos_sb[:])
            nc.vector.tensor_scalar(res[:], res[:], 1.0 / D, None, OP.mult)
            nc.sync.dma_start(out=out_d.ap(), in_=res[:])

    nc.finalize()
    return nc


_NC_CACHE = {}


def _get_nc(trivial_wb):
    if trivial_wb not in _NC_CACHE:
        _NC_CACHE[trivial_wb] = build_kernel(trivial_wb=trivial_wb)
    return _NC_CACHE[trivial_wb]


def make_in_maps(input_f, char_dic, ln_w, ln_b, target):
    input_f = np.ascontiguousarray(np.asarray(input_f, dtype=np.float32))
    char_dic = np.ascontiguousarray(np.asarray(char_dic, dtype=np.float32))
    ln_w = np.asarray(ln_w, dtype=np.float32)
    ln_b = np.asarray(ln_b, dtype=np.float32)
    labels = np.asarray(target).reshape(B, S)

    wbc = np.ascontiguousarray(np.broadcast_to(ln_w[None, :], (NCHAR, D)))
    bbc = np.ascontiguousarray(np.broadcast_to(ln_b[None, :], (NCHAR, D)))

    bpc = B // N_CORES
    in_maps = []
    for c in range(N_CORES):
        x_c = input_f[c * bpc:(c + 1) * bpc].reshape(TOK_PER_CORE, D)
        l_c = labels[c * bpc:(c + 1) * bpc].reshape(TOK_PER_CORE)
        # [tok] -> [p, tile*T]: token (i, p, f) lives at labT[p, i*T+f]
        l_t = np.ascontiguousarray(
            l_c.reshape(N_TILES, 128, T).transpose(1, 0, 2)
            .reshape(128, N_TILES * T).astype(ml_dtypes.bfloat16))
        in_maps.append({
            "x": np.ascontiguousarray(x_c),
            "lab": l_t,
            "char": char_dic,
            "wbc": wbc,
            "bbc": bbc,
        })
    return in_maps


def run(trace=False, **inputs):
    trivial_wb = bool(
        np.all(np.asarray(inputs["ln_w"], dtype=np.float32) == 1.0)
        and np.all(np.asarray(inputs["ln_b"], dtype=np.float32) == 0.0))
    nc = _get_nc(trivial_wb)
    in_maps = make_in_maps(**inputs)
    res = run_bass_kernel_spmd(nc, in_maps, core_ids=list(range(N_CORES)),
                               trace=trace)
    out = np.float32(res.results[0]["out"][0, 0])
    return out, res


def kernel(**inputs):
    out, _ = run(trace=False, **inputs)
    return np.array(out, dtype=np.float32)


if __name__ == "__main__":
    np.random.seed(0)
    input_f = np.random.randn(B, S, D).astype(np.float32)
    char_dic = np.random.randn(NCHAR, D).astype(np.float32)
    ln_w = np.ones(D, np.float32)
    ln_b = np.zeros(D, np.float32)
    target = np.random.randint(0, NCHAR, (B, S)).astype(np.int64)
    out = kernel(input_f=input_f, char_dic=char_dic, ln_w=ln_w,
                 ln_b=ln_b, target=target)
    print("kernel out:", out)


# revision 21
# speedup vs baseline: 1.0828x; 1.0334x over previous
"""Trainium2 Bass kernel for nn_ContrastLoss (LayerNorm + label segment-sum +
EMA codebook contrast loss), data-parallel over 8 NeuronCores.

Contract: kernel(**inputs) takes the FULL unsharded inputs
  input_f [128,1024,768] f32, char_dic [96,768] f32, ln_w [768] f32,
  ln_b [768] f32, target [128,1024] int64
and returns the full output (f32 scalar), matching reference.reference.

Strategy (hardcoded for the shapes above):
 - shard the batch dim over 8 cores: 16 batches = 16384 tokens per core
 - per core, stream 16 tiles of [128 partitions x 8 tokens x 768].
   The HBM->SBUF DMA casts f32->bf16 in flight (SWDGE), so no on-chip
   cast pass is needed.  Per half-tile (4 token groups):
     * ACT: one Square op [128,4,768] -> sq (bf16)
     * DVE: one multi-group reduce of xb -> sums[128,4], one of sq ->
       sumsq[128,4]
     * rstd = 1/sqrt((sumsq - sums^2/D)/D + eps) per token (small ops)
     * one-hot: (iota==label) as one bf16 tensor_tensor op, then
       scaled by rstd broadcast in a second op
     * TensorE: per token group 2 matmuls accumulate PSUM [96,384]x2;
       per half-tile one extra matmul ones^T @ oh_eq accumulates exact
       label counts into PSUM partition 96
   (ln_w/ln_b are folded out of the streaming loop entirely:
    tok_sums = (S - beta)*w + counts*b applied once at the end, where
    beta = row-mean of S recovers the -mu*rstd LayerNorm shift)
 - bf16 AllReduce of the [97,768] partial (S rows 0..95 | counts row 96)
   across cores; a tiny dummy AllReduce at kernel start absorbs the
   cross-core launch-skew barrier while the stream runs
 - tail math (group sums, positive term, EMA update, LayerNorm, negative
   term) computed replicated on every core; host reads core 0's scalar
"""

import os
import sys

for _p in ("/opt/trn_rl_repo",):
    if _p not in sys.path:
        sys.path.insert(0, _p)

import numpy as np
import ml_dtypes

import concourse.bass as bass
import concourse.bacc as bacc
import concourse.tile as tile
from concourse import mybir
from concourse.bass_utils import run_bass_kernel_spmd

F32 = mybir.dt.float32
BF16 = mybir.dt.bfloat16
AF = mybir.ActivationFunctionType
OP = mybir.AluOpType
AX = mybir.AxisListType

N_CORES = 8
B, S, D = 128, 1024, 768
NCHAR = 96
EPS = 1e-5
EMA = 0.1

TOK_PER_CORE = (B // N_CORES) * S          # 16384
T = 8                                      # tokens per partition per tile
H = T // 2                                 # tokens per half-tile
TILE_TOK = 128 * T                         # 1024 tokens per tile
N_TILES = TOK_PER_CORE // TILE_TOK         # 16


def build_kernel(n_tiles=N_TILES, trivial_wb=False):
    tok_per_core = n_tiles * TILE_TOK
    nc = bacc.Bacc("TRN2", target_bir_lowering=False, debug=False,
                   num_devices=N_CORES)

    x_d = nc.dram_tensor("x", [tok_per_core, D], F32, kind="ExternalInput")
    lab_d = nc.dram_tensor("lab", [128, n_tiles * T], BF16,
                           kind="ExternalInput")
    char_d = nc.dram_tensor("char", [NCHAR, D], F32, kind="ExternalInput")
    wbc_d = nc.dram_tensor("wbc", [NCHAR, D], F32, kind="ExternalInput")
    bbc_d = nc.dram_tensor("bbc", [NCHAR, D], F32, kind="ExternalInput")
    out_d = nc.dram_tensor("out", [1, 1], F32, kind="ExternalOutput")

    # constants embedded in the NEFF
    iota_np = np.tile(np.arange(NCHAR, dtype=np.float32), (128, T, 1))
    iota_d = nc.inline_tensor(iota_np.astype(ml_dtypes.bfloat16),
                              name="iotaH96")
    mask_np = np.ones((NCHAR, 1), dtype=np.float32)
    mask_np[0, 0] = 0.0
    mask_d = nc.inline_tensor(mask_np, name="maskrow")
    ones96_d = nc.inline_tensor(np.ones((NCHAR, 1), dtype=np.float32),
                                name="ones96")
    dmy_d = nc.inline_tensor(np.zeros((1, 128), dtype=np.float32),
                             name="dmyzero")

    # collective bounce buffers (bf16 payload: rows 0..95 = scaled segment
    # sums, row 96 cols 0..95 = exact label counts)
    cc_in = nc.dram_tensor("cc_in", [NCHAR + 1, D], BF16)
    cc_out = nc.dram_tensor("cc_out", [NCHAR + 1, D], BF16,
                            addr_space="Shared")
    dmy_out = nc.dram_tensor("dmy_out", [1, 128], F32, addr_space="Shared")

    x_r = x_d.ap().rearrange("(t p f) d -> t p f d",
                             t=n_tiles, p=128, f=T)

    with tile.TileContext(nc) as tc:
        with (
            nc.allow_low_precision(
                "bf16 row-sum outputs: DVE accumulates fp32 internally, "
                "only the final per-token value is downcast"),
            tc.tile_pool(name="consts", bufs=1) as consts,
            tc.tile_pool(name="xp", bufs=5) as xp,
            tc.tile_pool(name="sqp", bufs=3) as sqp,
            tc.tile_pool(name="trp", bufs=3) as trp,
            tc.tile_pool(name="stp", bufs=4) as stp,
            tc.tile_pool(name="ohp", bufs=3) as ohp,
            tc.tile_pool(name="tailp", bufs=1) as tailp,
            tc.tile_pool(name="psum", bufs=1, space="PSUM") as psp,
        ):
            # --- early dummy collective: pays the cross-core barrier /
            # bootstrap cost while the stream below runs ---
            nc.gpsimd.collective_compute(
                "AllReduce", OP.add,
                replica_groups=[list(range(N_CORES))],
                ins=[dmy_d.ap()], outs=[dmy_out.ap()],
            )

            # --- loop constants into SBUF ---
            iota_sb = consts.tile([128, T, NCHAR], BF16)
            nc.sync.dma_start(out=iota_sb[:], in_=iota_d.ap())
            ones128 = consts.tile([128, 1], BF16)
            nc.vector.memset(ones128[:], 1.0)
            eps128 = consts.tile([128, 1], F32)
            nc.vector.memset(eps128[:], EPS)
            eps96 = consts.tile([NCHAR, 1], F32)
            nc.vector.memset(eps96[:], EPS)
            # all labels for this core in one DMA: [p, tile*T]
            l_all = consts.tile([128, n_tiles * T], BF16)
            nc.sync.dma_start(out=l_all[:], in_=lab_d.ap())
            # tail constants (tiny, loaded while streaming)
            mask_sb = consts.tile([NCHAR, 1], F32)
            nc.sync.dma_start(out=mask_sb[:], in_=mask_d.ap())
            ones96_sb = consts.tile([NCHAR, 1], F32)
            nc.sync.dma_start(out=ones96_sb[:], in_=ones96_d.ap())
            char_sb = consts.tile([NCHAR, D], F32)
            nc.sync.dma_start(out=char_sb[:], in_=char_d.ap())
            if not trivial_wb:
                wbc_sb = consts.tile([NCHAR, D], F32)
                nc.sync.dma_start(out=wbc_sb[:], in_=wbc_d.ap())
                bbc_sb = consts.tile([NCHAR, D], F32)
                nc.sync.dma_start(out=bbc_sb[:], in_=bbc_d.ap())

            # --- PSUM accumulators for the streaming segment-sum ---
            psA = psp.tile([NCHAR, 384], F32)
            psB = psp.tile([NCHAR, 384], F32)
            cnt_ps = psp.tile([1, 384], F32)

            # --- streaming loop ---
            def add_tree(src, sums_out, n):
                """Row sums of src [128,n,768] via 2x-mode TT pairwise adds
                (DVE has no fast reduce: TENSOR_REDUCE is stuck at 1x)."""
                a1 = trp.tile([128, n, 384], BF16)
                nc.vector.tensor_tensor(a1[:], src[:, :, 0:384],
                                        src[:, :, 384:768], op=OP.add)
                a2 = trp.tile([128, n, 192], BF16)
                nc.vector.tensor_tensor(a2[:], a1[:, :, 0:192],
                                        a1[:, :, 192:384], op=OP.add)
                a3 = trp.tile([128, n, 96], BF16)
                nc.vector.tensor_tensor(a3[:], a2[:, :, 0:96],
                                        a2[:, :, 96:192], op=OP.add)
                nc.vector.reduce_sum(sums_out, a3[:], axis=AX.X)

            # 3-stage software pipeline: stage A(i) loads + row-stats,
            # stage B(i-1) finishes rstd, stage C(i-2) builds one-hots
            # and issues the matmuls.  The skew keeps every engine's
            # in-order instruction stream free of waits on ops issued in
            # the same iteration (ACT's sqrt / DVE's one-hot would
            # otherwise bubble both engines every half-tile).
            state = {}

            def stage_a(i):
                xb = xp.tile([128, T, D], BF16)
                sums = stp.tile([128, T], F32)
                sumsq = stp.tile([128, T], F32)
                for h in range(2):
                    g = slice(h * H, (h + 1) * H)
                    # f32 -> bf16 cast in flight (SWDGE)
                    nc.gpsimd.dma_start(out=xb[:, g, :],
                                        in_=x_r[i, :, g, :])
                add_tree(xb[:], sums[:], T)
                # row sums of squares, split within every tile so each
                # iteration loads ACT and DVE identically: the first 4
                # groups use ACT Square+accumulator, the last 4 use one
                # batched ACT Square + a DVE add-tree
                K = 5
                trash = sqp.tile([128, K, D], BF16)
                for t in range(K):
                    nc.scalar.activation(
                        trash[:, t, :], xb[:, t, :], AF.Square,
                        accum_out=sumsq[:, t:t + 1])
                sq = sqp.tile([128, T - K, D], BF16)
                nc.scalar.activation(sq[:], xb[:, K:T, :], AF.Square)
                add_tree(sq[:], sumsq[:, K:T], T - K)
                # rstd: 1/sqrt((sumsq - sums^2/D)/D + eps).  These live
                # at the end of the same iteration: their one cross-engine
                # wait (ACT sqrt on DVE xv) lands after ACT's squares
                # anyway, so no bubble.
                s2 = stp.tile([128, T], F32)
                nc.vector.scalar_tensor_tensor(
                    s2[:], sums[:], 1.0 / D, sums[:], OP.mult, OP.mult)
                xv = stp.tile([128, T], F32)
                nc.vector.tensor_sub(xv[:], sumsq[:], s2[:])
                stdv = stp.tile([128, T], F32)
                nc.scalar.activation(stdv[:], xv[:], AF.Sqrt,
                                     bias=eps128[:], scale=1.0 / D)
                rstd = stp.tile([128, T], F32)
                nc.vector.reciprocal(rstd[:], stdv[:])
                state[i] = {"xb": xb, "rstd": rstd}

            def stage_c(i):
                st = state.pop(i)
                xb, rstd = st["xb"], st["rstd"]
                oh_eq = ohp.tile([128, T, NCHAR], BF16)
                lab_b = l_all[:, i * T:(i + 1) * T]
                nc.vector.tensor_tensor(
                    oh_eq[:], iota_sb[:],
                    lab_b[:, :, None].broadcast_to((128, T, NCHAR)),
                    op=OP.is_equal)
                oh = ohp.tile([128, T, NCHAR], BF16)
                nc.vector.tensor_tensor(
                    oh[:], oh_eq[:],
                    rstd[:, :, None].broadcast_to((128, T, NCHAR)),
                    op=OP.mult)

                for t in range(T):
                    st0 = i == 0 and t == 0
                    sp0 = i == n_tiles - 1 and t == T - 1
                    nc.tensor.matmul(psA[:], oh[:, t, :],
                                     xb[:, t, 0:384],
                                     start=st0, stop=sp0)
                    nc.tensor.matmul(psB[:], oh[:, t, :],
                                     xb[:, t, 384:D],
                                     start=st0, stop=sp0)
                first, last = i == 0, i == n_tiles - 1
                nc.tensor.matmul(cnt_ps[:], ones128[:],
                                 oh_eq[:, 0:H, :],
                                 start=first, stop=last)
                nc.tensor.matmul(cnt_ps[:], ones128[:],
                                 oh_eq[:, H:T, :],
                                 start=False if not first else False,
                                 stop=last)

            # issue order C(i-1), A(i): the one-hot TTs lead each
            # iteration's DVE stream, so TensorE gets its matmul inputs
            # at the start of the period instead of after the next
            # tile's add-trees
            for i in range(n_tiles + 1):
                if 0 <= i - 1 < n_tiles:
                    stage_c(i - 1)
                if i < n_tiles:
                    stage_a(i)

            # --- local partials -> bf16 -> DRAM -> AllReduce ---
            acc = tailp.tile([NCHAR, D], BF16)
            nc.vector.tensor_copy(acc[:, 0:384], psA[:])
            nc.vector.tensor_copy(acc[:, 384:D], psB[:])
            # counts: cnt_ps holds [u*96+c] = counts of groups (u, 4+u);
            # fold the 4 u-slices, then ship as payload row 96
            cnt_pay = tailp.tile([1, D], BF16)
            nc.vector.memset(cnt_pay[:], 0.0)
            cnt_red = tailp.tile([1, NCHAR], F32)
            nc.vector.reduce_sum(
                cnt_red[:],
                cnt_ps[:].rearrange("p (t c) -> p c t", c=NCHAR),
                axis=AX.X)
            nc.vector.tensor_copy(cnt_pay[:, 0:NCHAR], cnt_red[:])
            nc.sync.dma_start(out=cc_in.ap()[0:NCHAR, :], in_=acc[:])
            nc.sync.dma_start(out=cc_in.ap()[NCHAR:NCHAR + 1, :],
                              in_=cnt_pay[:])
            nc.gpsimd.collective_compute(
                "AllReduce", OP.add,
                replica_groups=[list(range(N_CORES))],
                ins=[cc_in.ap()], outs=[cc_out.ap()],
            )
            cntb = tailp.tile([NCHAR, 1], BF16)
            nc.scalar.dma_start(
                out=cntb[:],
                in_=cc_out.ap()[NCHAR:NCHAR + 1, 0:NCHAR].rearrange(
                    "p c -> c p"))
            redb = tailp.tile([NCHAR, D], BF16)
            nc.sync.dma_start(out=redb[:], in_=cc_out.ap()[0:NCHAR, :])
            cnt = tailp.tile([NCHAR, 1], F32)
            nc.vector.tensor_copy(cnt[:], cntb[:])

            # beta_i = mean_d S[i, d]  (the LayerNorm -mu*rstd correction
            # folds into a row-mean of the scaled segment sums)
            rs = tailp.tile([NCHAR, 1], F32)
            nc.vector.reduce_sum(rs[:], redb[:], axis=AX.X)
            nb = tailp.tile([NCHAR, 1], F32)
            nc.vector.tensor_scalar(nb[:], rs[:], -1.0 / D, None, OP.mult)
            # group_sum = char + (S - beta)*w + counts*b
            group = tailp.tile([NCHAR, D], F32)
            if trivial_wb:
                nc.vector.scalar_tensor_tensor(group[:], redb[:], nb[:],
                                               char_sb[:], OP.add, OP.add)
            else:
                tmp1 = tailp.tile([NCHAR, D], F32)
                nc.vector.scalar_tensor_tensor(tmp1[:], bbc_sb[:], cnt[:],
                                               char_sb[:], OP.mult, OP.add)
                nc.vector.scalar_tensor_tensor(group[:], redb[:], nb[:],
                                               wbc_sb[:], OP.add, OP.mult)
                nc.vector.tensor_add(group[:], group[:], tmp1[:])

            # positive = sum(group^2) (divide by D at the very end)
            sqg = tailp.tile([NCHAR, D], F32)
            pos_col = tailp.tile([NCHAR, 1], F32)
            nc.scalar.activation(sqg[:], group[:], AF.Square,
                                 accum_out=pos_col[:])
            pos_ps = psp.tile([1, 1], F32)
            nc.tensor.matmul(pos_ps[:], ones96_sb[:], pos_col[:],
                             start=True, stop=True)
            pos_sb = tailp.tile([1, 1], F32)
            nc.vector.tensor_copy(pos_sb[:], pos_ps[:])

            # EMA update: new_char = char + 0.1 * group/(counts+1); row 0 kept
            cnt1 = tailp.tile([NCHAR, 1], F32)
            nc.vector.tensor_scalar(cnt1[:], cnt[:], 1.0, None, OP.add)
            invc = tailp.tile([NCHAR, 1], F32)
            nc.vector.reciprocal(invc[:], cnt1[:])
            ema = tailp.tile([NCHAR, D], F32)
            nc.vector.tensor_scalar(ema[:], group[:], invc[:], EMA,
                                    OP.mult, OP.mult)
            newc = tailp.tile([NCHAR, D], F32)
            nc.vector.tensor_add(newc[:], char_sb[:], ema[:])
            nc.vector.tensor_copy(newc[0:1, :], char_sb[0:1, :])

            # LayerNorm(new_char) with w/b
            bn2 = tailp.tile([NCHAR, 2, 6], F32)
            for gidx in range(2):
                nc.vector.bn_stats(bn2[:, gidx, :],
                                   newc[:, gidx * 384:(gidx + 1) * 384])
            st2 = tailp.tile([NCHAR, 2], F32)
            nc.vector.bn_aggr(st2[:], bn2[:])
            std2 = tailp.tile([NCHAR, 1], F32)
            nc.scalar.activation(std2[:], st2[:, 1:2], AF.Sqrt,
                                 bias=eps96[:], scale=1.0)
            rstd2 = tailp.tile([NCHAR, 1], F32)
            nc.vector.reciprocal(rstd2[:], std2[:])
            nmr2 = tailp.tile([NCHAR, 1], F32)
            nc.vector.scalar_tensor_tensor(nmr2[:], st2[:, 0:1], -1.0,
                                           rstd2[:], OP.mult, OP.mult)
            nrm = tailp.tile([NCHAR, D], F32)
            nc.scalar.activation(nrm[:], newc[:], AF.Identity,
                                 bias=nmr2[:], scale=rstd2[:])
            if trivial_wb:
                fin = nrm
            else:
                fin = tailp.tile([NCHAR, D], F32)
                nc.vector.tensor_mul(fin[:], nrm[:], wbc_sb[:])
                nc.vector.tensor_add(fin[:], fin[:], bbc_sb[:])

            # s = sum over rows 1..95 -> [1,768]; negative = sum(s^2)
            sA = psp.tile([1, 384], F32)
            sB = psp.tile([1, 384], F32)
            nc.tensor.matmul(sA[:], mask_sb[:], fin[:, 0:384],
                             start=True, stop=True)
            nc.tensor.matmul(sB[:], mask_sb[:], fin[:, 384:D],
                             start=True, stop=True)
            sqA = tailp.tile([1, 384], F32)
            sqB = tailp.tile([1, 384], F32)
            negA = tailp.tile([1, 1], F32)
            negB = tailp.tile([1, 1], F32)
            nc.scalar.activation(sqA[:], sA[:], AF.Square, accum_out=negA[:])
            nc.scalar.activation(sqB[:], sB[:], AF.Square, accum_out=negB[:])

            res = tailp.tile([1, 1], F32)
            nc.vector.tensor_add(res[:], negA[:], negB[:])
            nc.vector.tensor_sub(res[:], res[:], pos_sb[:])
            nc.vector.tensor_scalar(res[:], res[:], 1.0 / D, None, OP.mult)
            nc.sync.dma_start(out=out_d.ap(), in_=res[:])

    nc.finalize()
    return nc


_NC_CACHE = {}


def _get_nc(trivial_wb):
    if trivial_wb not in _NC_CACHE:
        _NC_CACHE[trivial_wb] = build_kernel(trivial_wb=trivial_wb)
    return _NC_CACHE[trivial_wb]


def make_in_maps(input_f, char_dic, ln_w, ln_b, target):
    input_f = np.ascontiguousarray(np.asarray(input_f, dtype=np.float32))
    char_dic = np.ascontiguousarray(np.asarray(char_dic, dtype=np.float32))
    ln_w = np.asarray(ln_w, dtype=np.float32)
    ln_b = np.asarray(ln_b, dtype=np.float32)
    labels = np.asarray(target).reshape(B, S)

    wbc = np.ascontiguousarray(np.broadcast_to(ln_w[None, :], (NCHAR, D)))
    bbc = np.ascontiguousarray(np.broadcast_to(ln_b[None, :], (NCHAR, D)))

    bpc = B // N_CORES
    in_maps = []
    for c in range(N_CORES):
        x_c = input_f[c * bpc:(c + 1) * bpc].reshape(TOK_PER_CORE, D)
        l_c = labels[c * bpc:(c + 1) * bpc].reshape(TOK_PER_CORE)
        # [tok] -> [p, tile*T]: token (i, p, f) lives at labT[p, i*T+f]
        l_t = np.ascontiguousarray(
            l_c.reshape(N_TILES, 128, T).transpose(1, 0, 2)
            .reshape(128, N_TILES * T).astype(ml_dtypes.bfloat16))
        in_maps.append({
            "x": np.ascontiguousarray(x_c),
            "lab": l_t,
            "char": char_dic,
            "wbc": wbc,
            "bbc": bbc,
        })
    return in_maps


def run(trace=False, **inputs):
    trivial_wb = bool(
        np.all(np.asarray(inputs["ln_w"], dtype=np.float32) == 1.0)
        and np.all(np.asarray(inputs["ln_b"], dtype=np.float32) == 0.0))
    nc = _get_nc(trivial_wb)
    in_maps = make_in_maps(**inputs)
    res = run_bass_kernel_spmd(nc, in_maps, core_ids=list(range(N_CORES)),
                               trace=trace)
    out = np.float32(res.results[0]["out"][0, 0])
    return out, res


def kernel(**inputs):
    out, _ = run(trace=False, **inputs)
    return np.array(out, dtype=np.float32)


if __name__ == "__main__":
    np.random.seed(0)
    input_f = np.random.randn(B, S, D).astype(np.float32)
    char_dic = np.random.randn(NCHAR, D).astype(np.float32)
    ln_w = np.ones(D, np.float32)
    ln_b = np.zeros(D, np.float32)
    target = np.random.randint(0, NCHAR, (B, S)).astype(np.int64)
    out = kernel(input_f=input_f, char_dic=char_dic, ln_w=ln_w,
                 ln_b=ln_b, target=target)
    print("kernel out:", out)


# revision 22
# speedup vs baseline: 1.1377x; 1.0507x over previous
"""Trainium2 Bass kernel for nn_ContrastLoss (LayerNorm + label segment-sum +
EMA codebook contrast loss), data-parallel over 8 NeuronCores.

Contract: kernel(**inputs) takes the FULL unsharded inputs
  input_f [128,1024,768] f32, char_dic [96,768] f32, ln_w [768] f32,
  ln_b [768] f32, target [128,1024] int64
and returns the full output (f32 scalar), matching reference.reference.

Strategy (hardcoded for the shapes above):
 - shard the batch dim over 8 cores: 16 batches = 16384 tokens per core
 - per core, stream 16 tiles of [128 partitions x 8 tokens x 768].
   The HBM->SBUF DMA casts f32->bf16 in flight (SWDGE), so no on-chip
   cast pass is needed.  Per half-tile (4 token groups):
     * ACT: one Square op [128,4,768] -> sq (bf16)
     * DVE: one multi-group reduce of xb -> sums[128,4], one of sq ->
       sumsq[128,4]
     * rstd = 1/sqrt((sumsq - sums^2/D)/D + eps) per token (small ops)
     * one-hot: (iota==label) as one bf16 tensor_tensor op, then
       scaled by rstd broadcast in a second op
     * TensorE: per token group 2 matmuls accumulate PSUM [96,384]x2;
       per half-tile one extra matmul ones^T @ oh_eq accumulates exact
       label counts into PSUM partition 96
   (ln_w/ln_b are folded out of the streaming loop entirely:
    tok_sums = (S - beta)*w + counts*b applied once at the end, where
    beta = row-mean of S recovers the -mu*rstd LayerNorm shift)
 - bf16 AllReduce of the [97,768] partial (S rows 0..95 | counts row 96)
   across cores; a tiny dummy AllReduce at kernel start absorbs the
   cross-core launch-skew barrier while the stream runs
 - tail math (group sums, positive term, EMA update, LayerNorm, negative
   term) computed replicated on every core; host reads core 0's scalar
"""

import os
import sys

for _p in ("/opt/trn_rl_repo",):
    if _p not in sys.path:
        sys.path.insert(0, _p)

import numpy as np
import ml_dtypes

import concourse.bass as bass
import concourse.bacc as bacc
import concourse.tile as tile
from concourse import mybir
from concourse.bass_utils import run_bass_kernel_spmd

F32 = mybir.dt.float32
BF16 = mybir.dt.bfloat16
AF = mybir.ActivationFunctionType
OP = mybir.AluOpType
AX = mybir.AxisListType

N_CORES = 8
B, S, D = 128, 1024, 768
NCHAR = 96
EPS = 1e-5
EMA = 0.1

TOK_PER_CORE = (B // N_CORES) * S          # 16384
T = 8                                      # tokens per partition per tile
H = T // 2                                 # tokens per half-tile
TILE_TOK = 128 * T                         # 1024 tokens per tile
N_TILES = TOK_PER_CORE // TILE_TOK         # 16


def build_kernel(n_tiles=N_TILES, trivial_wb=False):
    tok_per_core = n_tiles * TILE_TOK
    nc = bacc.Bacc("TRN2", target_bir_lowering=False, debug=False,
                   num_devices=N_CORES)

    x_d = nc.dram_tensor("x", [tok_per_core, D], F32, kind="ExternalInput")
    lab_d = nc.dram_tensor("lab", [128, n_tiles * T], BF16,
                           kind="ExternalInput")
    char_d = nc.dram_tensor("char", [NCHAR, D], F32, kind="ExternalInput")
    wbc_d = nc.dram_tensor("wbc", [NCHAR, D], F32, kind="ExternalInput")
    bbc_d = nc.dram_tensor("bbc", [NCHAR, D], F32, kind="ExternalInput")
    out_d = nc.dram_tensor("out", [1, 1], F32, kind="ExternalOutput")

    # constants embedded in the NEFF
    iota_np = np.tile(np.arange(NCHAR, dtype=np.float32), (128, T, 1))
    iota_d = nc.inline_tensor(iota_np.astype(ml_dtypes.bfloat16),
                              name="iotaH96")
    mask_np = np.ones((NCHAR, 1), dtype=np.float32)
    mask_np[0, 0] = 0.0
    mask_d = nc.inline_tensor(mask_np, name="maskrow")
    ones96_d = nc.inline_tensor(np.ones((NCHAR, 1), dtype=np.float32),
                                name="ones96")
    dmy_d = nc.inline_tensor(np.zeros((1, 128), dtype=np.float32),
                             name="dmyzero")

    # collective bounce buffers (bf16 payload: rows 0..95 = scaled segment
    # sums, row 96 cols 0..95 = exact label counts)
    cc_in = nc.dram_tensor("cc_in", [NCHAR + 1, D], BF16)
    cc_out = nc.dram_tensor("cc_out", [NCHAR + 1, D], BF16,
                            addr_space="Shared")
    dmy_out = nc.dram_tensor("dmy_out", [1, 128], F32, addr_space="Shared")

    x_r = x_d.ap().rearrange("(t p f) d -> t p f d",
                             t=n_tiles, p=128, f=T)

    with tile.TileContext(nc) as tc:
        with (
            nc.allow_low_precision(
                "bf16 row-sum outputs: DVE accumulates fp32 internally, "
                "only the final per-token value is downcast"),
            tc.tile_pool(name="consts", bufs=1) as consts,
            tc.tile_pool(name="xp", bufs=5) as xp,
            tc.tile_pool(name="sqp", bufs=3) as sqp,
            tc.tile_pool(name="trp", bufs=3) as trp,
            tc.tile_pool(name="stp", bufs=4) as stp,
            tc.tile_pool(name="ohp", bufs=3) as ohp,
            tc.tile_pool(name="tailp", bufs=1) as tailp,
            tc.tile_pool(name="psum", bufs=1, space="PSUM") as psp,
        ):
            # --- early dummy collective: pays the cross-core barrier /
            # bootstrap cost while the stream below runs ---
            nc.gpsimd.collective_compute(
                "AllReduce", OP.add,
                replica_groups=[list(range(N_CORES))],
                ins=[dmy_d.ap()], outs=[dmy_out.ap()],
            )

            # --- loop constants into SBUF ---
            iota_sb = consts.tile([128, T, NCHAR], BF16)
            nc.sync.dma_start(out=iota_sb[:], in_=iota_d.ap())
            ones128 = consts.tile([128, 1], BF16)
            nc.vector.memset(ones128[:], 1.0)
            eps128 = consts.tile([128, 1], F32)
            nc.vector.memset(eps128[:], EPS)
            eps96 = consts.tile([NCHAR, 1], F32)
            nc.vector.memset(eps96[:], EPS)
            # all labels for this core in one DMA: [p, tile*T]
            l_all = consts.tile([128, n_tiles * T], BF16)
            nc.sync.dma_start(out=l_all[:], in_=lab_d.ap())
            # tail constants (tiny, loaded while streaming)
            mask_sb = consts.tile([NCHAR, 1], F32)
            nc.sync.dma_start(out=mask_sb[:], in_=mask_d.ap())
            ones96_sb = consts.tile([NCHAR, 1], F32)
            nc.sync.dma_start(out=ones96_sb[:], in_=ones96_d.ap())
            char_sb = consts.tile([NCHAR, D], F32)
            nc.sync.dma_start(out=char_sb[:], in_=char_d.ap())
            if not trivial_wb:
                wbc_sb = consts.tile([NCHAR, D], F32)
                nc.sync.dma_start(out=wbc_sb[:], in_=wbc_d.ap())
                bbc_sb = consts.tile([NCHAR, D], F32)
                nc.sync.dma_start(out=bbc_sb[:], in_=bbc_d.ap())

            # --- PSUM accumulators for the streaming segment-sum ---
            psA = psp.tile([NCHAR, 384], F32)
            psB = psp.tile([NCHAR, 384], F32)
            cnt_ps = psp.tile([1, 384], F32)

            # --- streaming loop ---
            def add_tree(src, sums_out, n):
                """Row sums of src [128,n,768] via 2x-mode TT pairwise adds
                (DVE has no fast reduce: TENSOR_REDUCE is stuck at 1x)."""
                a1 = trp.tile([128, n, 384], BF16)
                nc.vector.tensor_tensor(a1[:], src[:, :, 0:384],
                                        src[:, :, 384:768], op=OP.add)
                a2 = trp.tile([128, n, 192], BF16)
                nc.vector.tensor_tensor(a2[:], a1[:, :, 0:192],
                                        a1[:, :, 192:384], op=OP.add)
                a3 = trp.tile([128, n, 96], BF16)
                nc.vector.tensor_tensor(a3[:], a2[:, :, 0:96],
                                        a2[:, :, 96:192], op=OP.add)
                nc.vector.reduce_sum(sums_out, a3[:], axis=AX.X)

            # 3-stage software pipeline: stage A(i) loads + row-stats,
            # stage B(i-1) finishes rstd, stage C(i-2) builds one-hots
            # and issues the matmuls.  The skew keeps every engine's
            # in-order instruction stream free of waits on ops issued in
            # the same iteration (ACT's sqrt / DVE's one-hot would
            # otherwise bubble both engines every half-tile).
            state = {}

            def stage_a(i):
                xb = xp.tile([128, T, D], BF16)
                sums = stp.tile([128, T], F32)
                sumsq = stp.tile([128, T], F32)
                for h in range(2):
                    g = slice(h * H, (h + 1) * H)
                    # f32 -> bf16 cast in flight (SWDGE)
                    nc.gpsimd.dma_start(out=xb[:, g, :],
                                        in_=x_r[i, :, g, :])
                add_tree(xb[:], sums[:], T)
                # second half of the previous tile's one-hot + matmuls,
                # issued mid-iteration so TensorE never idles past the
                # HAM re-throttle window
                if i - 1 >= 0:
                    stage_c_half(i - 1, 1)
                # row sums of squares, split within every tile so each
                # iteration loads ACT and DVE identically: the first 4
                # groups use ACT Square+accumulator, the last 4 use one
                # batched ACT Square + a DVE add-tree
                K = 5
                trash = sqp.tile([128, K, D], BF16)
                for t in range(K):
                    nc.scalar.activation(
                        trash[:, t, :], xb[:, t, :], AF.Square,
                        accum_out=sumsq[:, t:t + 1])
                sq = sqp.tile([128, T - K, D], BF16)
                nc.scalar.activation(sq[:], xb[:, K:T, :], AF.Square)
                add_tree(sq[:], sumsq[:, K:T], T - K)
                # rstd: 1/sqrt((sumsq - sums^2/D)/D + eps).  These live
                # at the end of the same iteration: their one cross-engine
                # wait (ACT sqrt on DVE xv) lands after ACT's squares
                # anyway, so no bubble.
                s2 = stp.tile([128, T], F32)
                nc.vector.scalar_tensor_tensor(
                    s2[:], sums[:], 1.0 / D, sums[:], OP.mult, OP.mult)
                xv = stp.tile([128, T], F32)
                nc.vector.tensor_sub(xv[:], sumsq[:], s2[:])
                stdv = stp.tile([128, T], F32)
                nc.scalar.activation(stdv[:], xv[:], AF.Sqrt,
                                     bias=eps128[:], scale=1.0 / D)
                rstd = stp.tile([128, T], F32)
                nc.vector.reciprocal(rstd[:], stdv[:])
                state[i] = {"xb": xb, "rstd": rstd}

            def stage_c_half(i, h):
                st = state[i]
                xb, rstd = st["xb"], st["rstd"]
                g = slice(h * H, (h + 1) * H)
                oh_eq = ohp.tile([128, H, NCHAR], BF16)
                lab_b = l_all[:, i * T + h * H:i * T + (h + 1) * H]
                nc.vector.tensor_tensor(
                    oh_eq[:], iota_sb[:, 0:H, :],
                    lab_b[:, :, None].broadcast_to((128, H, NCHAR)),
                    op=OP.is_equal)
                oh = ohp.tile([128, H, NCHAR], BF16)
                nc.vector.tensor_tensor(
                    oh[:], oh_eq[:],
                    rstd[:, g][:, :, None].broadcast_to((128, H, NCHAR)),
                    op=OP.mult)

                for tt in range(H):
                    t = h * H + tt
                    st0 = i == 0 and t == 0
                    sp0 = i == n_tiles - 1 and t == T - 1
                    nc.tensor.matmul(psA[:], oh[:, tt, :],
                                     xb[:, t, 0:384],
                                     start=st0, stop=sp0)
                    nc.tensor.matmul(psB[:], oh[:, tt, :],
                                     xb[:, t, 384:D],
                                     start=st0, stop=sp0)
                nc.tensor.matmul(cnt_ps[:], ones128[:], oh_eq[:],
                                 start=(i == 0 and h == 0),
                                 stop=(i == n_tiles - 1 and h == 1))
                if h == 1:
                    state.pop(i)

            # issue order C(i-1), A(i): the one-hot TTs lead each
            # iteration's DVE stream, so TensorE gets its matmul inputs
            # at the start of the period instead of after the next
            # tile's add-trees
            for i in range(n_tiles + 1):
                if 0 <= i - 1 < n_tiles:
                    stage_c_half(i - 1, 0)
                if i < n_tiles:
                    stage_a(i)
                else:
                    stage_c_half(i - 1, 1)

            # --- local partials -> bf16 -> DRAM -> AllReduce ---
            acc = tailp.tile([NCHAR, D], BF16)
            nc.vector.tensor_copy(acc[:, 0:384], psA[:])
            nc.vector.tensor_copy(acc[:, 384:D], psB[:])
            # counts: cnt_ps holds [u*96+c] = counts of groups (u, 4+u);
            # fold the 4 u-slices, then ship as payload row 96
            cnt_pay = tailp.tile([1, D], BF16)
            nc.vector.memset(cnt_pay[:], 0.0)
            cnt_red = tailp.tile([1, NCHAR], F32)
            nc.vector.reduce_sum(
                cnt_red[:],
                cnt_ps[:].rearrange("p (t c) -> p c t", c=NCHAR),
                axis=AX.X)
            nc.vector.tensor_copy(cnt_pay[:, 0:NCHAR], cnt_red[:])
            nc.sync.dma_start(out=cc_in.ap()[0:NCHAR, :], in_=acc[:])
            nc.sync.dma_start(out=cc_in.ap()[NCHAR:NCHAR + 1, :],
                              in_=cnt_pay[:])
            nc.gpsimd.collective_compute(
                "AllReduce", OP.add,
                replica_groups=[list(range(N_CORES))],
                ins=[cc_in.ap()], outs=[cc_out.ap()],
            )
            cntb = tailp.tile([NCHAR, 1], BF16)
            nc.scalar.dma_start(
                out=cntb[:],
                in_=cc_out.ap()[NCHAR:NCHAR + 1, 0:NCHAR].rearrange(
                    "p c -> c p"))
            redb = tailp.tile([NCHAR, D], BF16)
            nc.sync.dma_start(out=redb[:], in_=cc_out.ap()[0:NCHAR, :])
            cnt = tailp.tile([NCHAR, 1], F32)
            nc.vector.tensor_copy(cnt[:], cntb[:])

            # beta_i = mean_d S[i, d]  (the LayerNorm -mu*rstd correction
            # folds into a row-mean of the scaled segment sums)
            rs = tailp.tile([NCHAR, 1], F32)
            nc.vector.reduce_sum(rs[:], redb[:], axis=AX.X)
            nb = tailp.tile([NCHAR, 1], F32)
            nc.vector.tensor_scalar(nb[:], rs[:], -1.0 / D, None, OP.mult)
            # group_sum = char + (S - beta)*w + counts*b
            group = tailp.tile([NCHAR, D], F32)
            if trivial_wb:
                nc.vector.scalar_tensor_tensor(group[:], redb[:], nb[:],
                                               char_sb[:], OP.add, OP.add)
            else:
                tmp1 = tailp.tile([NCHAR, D], F32)
                nc.vector.scalar_tensor_tensor(tmp1[:], bbc_sb[:], cnt[:],
                                               char_sb[:], OP.mult, OP.add)
                nc.vector.scalar_tensor_tensor(group[:], redb[:], nb[:],
                                               wbc_sb[:], OP.add, OP.mult)
                nc.vector.tensor_add(group[:], group[:], tmp1[:])

            # positive = sum(group^2) (divide by D at the very end)
            sqg = tailp.tile([NCHAR, D], F32)
            pos_col = tailp.tile([NCHAR, 1], F32)
            nc.scalar.activation(sqg[:], group[:], AF.Square,
                                 accum_out=pos_col[:])
            pos_ps = psp.tile([1, 1], F32)
            nc.tensor.matmul(pos_ps[:], ones96_sb[:], pos_col[:],
                             start=True, stop=True)
            pos_sb = tailp.tile([1, 1], F32)
            nc.vector.tensor_copy(pos_sb[:], pos_ps[:])

            # EMA update: new_char = char + 0.1 * group/(counts+1); row 0 kept
            cnt1 = tailp.tile([NCHAR, 1], F32)
            nc.vector.tensor_scalar(cnt1[:], cnt[:], 1.0, None, OP.add)
            invc = tailp.tile([NCHAR, 1], F32)
            nc.vector.reciprocal(invc[:], cnt1[:])
            ema = tailp.tile([NCHAR, D], F32)
            nc.vector.tensor_scalar(ema[:], group[:], invc[:], EMA,
                                    OP.mult, OP.mult)
            newc = tailp.tile([NCHAR, D], F32)
            nc.vector.tensor_add(newc[:], char_sb[:], ema[:])
            nc.vector.tensor_copy(newc[0:1, :], char_sb[0:1, :])

            # LayerNorm(new_char) with w/b
            bn2 = tailp.tile([NCHAR, 2, 6], F32)
            for gidx in range(2):
                nc.vector.bn_stats(bn2[:, gidx, :],
                                   newc[:, gidx * 384:(gidx + 1) * 384])
            st2 = tailp.tile([NCHAR, 2], F32)
            nc.vector.bn_aggr(st2[:], bn2[:])
            std2 = tailp.tile([NCHAR, 1], F32)
            nc.scalar.activation(std2[:], st2[:, 1:2], AF.Sqrt,
                                 bias=eps96[:], scale=1.0)
            rstd2 = tailp.tile([NCHAR, 1], F32)
            nc.vector.reciprocal(rstd2[:], std2[:])
            nmr2 = tailp.tile([NCHAR, 1], F32)
            nc.vector.scalar_tensor_tensor(nmr2[:], st2[:, 0:1], -1.0,
                                           rstd2[:], OP.mult, OP.mult)
            nrm = tailp.tile([NCHAR, D], F32)
            nc.scalar.activation(nrm[:], newc[:], AF.Identity,
                                 bias=nmr2[:], scale=rstd2[:])
            if trivial_wb:
                fin = nrm
            else:
                fin = tailp.tile([NCHAR, D], F32)
                nc.vector.tensor_mul(fin[:], nrm[:], wbc_sb[:])
                nc.vector.tensor_add(fin[:], fin[:], bbc_sb[:])

            # s = sum over rows 1..95 -> [1,768]; negative = sum(s^2)
            sA = psp.tile([1, 384], F32)
            sB = psp.tile([1, 384], F32)
            nc.tensor.matmul(sA[:], mask_sb[:], fin[:, 0:384],
                             start=True, stop=True)
            nc.tensor.matmul(sB[:], mask_sb[:], fin[:, 384:D],
                             start=True, stop=True)
            sqA = tailp.tile([1, 384], F32)
            sqB = tailp.tile([1, 384], F32)
            negA = tailp.tile([1, 1], F32)
            negB = tailp.tile([1, 1], F32)
            nc.scalar.activation(sqA[:], sA[:], AF.Square, accum_out=negA[:])
            nc.scalar.activation(sqB[:], sB[:], AF.Square, accum_out=negB[:])

            res = tailp.tile([1, 1], F32)
            nc.vector.tensor_add(res[:], negA[:], negB[:])
            nc.vector.tensor_sub(res[:], res[:], pos_sb[:])
            nc.vector.tensor_scalar(res[:], res[:], 1.0 / D, None, OP.mult)
            nc.sync.dma_start(out=out_d.ap(), in_=res[:])

    nc.finalize()
    return nc


_NC_CACHE = {}


def _get_nc(trivial_wb):
    if trivial_wb not in _NC_CACHE:
        _NC_CACHE[trivial_wb] = build_kernel(trivial_wb=trivial_wb)
    return _NC_CACHE[trivial_wb]


def make_in_maps(input_f, char_dic, ln_w, ln_b, target):
    input_f = np.ascontiguousarray(np.asarray(input_f, dtype=np.float32))
    char_dic = np.ascontiguousarray(np.asarray(char_dic, dtype=np.float32))
    ln_w = np.asarray(ln_w, dtype=np.float32)
    ln_b = np.asarray(ln_b, dtype=np.float32)
    labels = np.asarray(target).reshape(B, S)

    wbc = np.ascontiguousarray(np.broadcast_to(ln_w[None, :], (NCHAR, D)))
    bbc = np.ascontiguousarray(np.broadcast_to(ln_b[None, :], (NCHAR, D)))

    bpc = B // N_CORES
    in_maps = []
    for c in range(N_CORES):
        x_c = input_f[c * bpc:(c + 1) * bpc].reshape(TOK_PER_CORE, D)
        l_c = labels[c * bpc:(c + 1) * bpc].reshape(TOK_PER_CORE)
        # [tok] -> [p, tile*T]: token (i, p, f) lives at labT[p, i*T+f]
        l_t = np.ascontiguousarray(
            l_c.reshape(N_TILES, 128, T).transpose(1, 0, 2)
            .reshape(128, N_TILES * T).astype(ml_dtypes.bfloat16))
        in_maps.append({
            "x": np.ascontiguousarray(x_c),
            "lab": l_t,
            "char": char_dic,
            "wbc": wbc,
            "bbc": bbc,
        })
    return in_maps


def run(trace=False, **inputs):
    trivial_wb = bool(
        np.all(np.asarray(inputs["ln_w"], dtype=np.float32) == 1.0)
        and np.all(np.asarray(inputs["ln_b"], dtype=np.float32) == 0.0))
    nc = _get_nc(trivial_wb)
    in_maps = make_in_maps(**inputs)
    res = run_bass_kernel_spmd(nc, in_maps, core_ids=list(range(N_CORES)),
                               trace=trace)
    out = np.float32(res.results[0]["out"][0, 0])
    return out, res


def kernel(**inputs):
    out, _ = run(trace=False, **inputs)
    return np.array(out, dtype=np.float32)


if __name__ == "__main__":
    np.random.seed(0)
    input_f = np.random.randn(B, S, D).astype(np.float32)
    char_dic = np.random.randn(NCHAR, D).astype(np.float32)
    ln_w = np.ones(D, np.float32)
    ln_b = np.zeros(D, np.float32)
    target = np.random.randint(0, NCHAR, (B, S)).astype(np.int64)
    out = kernel(input_f=input_f, char_dic=char_dic, ln_w=ln_w,
                 ln_b=ln_b, target=target)
    print("kernel out:", out)
